# revision 21
# baseline (speedup 1.0000x reference)
"""ConvNeXt block kernel for Trainium2 (8 NeuronCores, data-parallel over batch).

Reference semantics (per image):
  y = x + gamma * ( GELU( LN(dwconv7x7(x) + dw_b) @ w1 + b1 ) @ w2 + b2 )
with LN over channels, exact (erf) GELU, NCHW in/out.

Distribution: batch 16 -> 2 images per core across 8 cores. No collectives.

Per-core layout: channels on partitions (3 blocks of 128), pixels on the free
dim.  The depthwise 7x7 conv runs on a W-padded bf16 image and is split by
output rows across three engine strategies:
  rows  0..24: TensorE — 49 diagonal-weight matmuls per 448-pixel PSUM chunk
  rows 24..40: ScalarE products (per-partition scale) + VectorE adds
  rows 40..56: VectorE tensor_scalar products (4x mode) + tensor_tensor adds
The MLP runs on the tensor engine in bf16 with LN folded in: stats via a
ones-matmul, rstd via a magic-constant Newton rsqrt on the vector engine in a
[112,28] transposed layout (DRAM bounce), the mean correction as an extra k=1
matmul row, b1 via the GELU activation bias.  The residual is re-loaded from
DRAM in fp32 and added on the vector engine, so the dominant output term never
passes through bf16.
"""

import sys

sys.path.insert(0, "/opt/trn_rl_repo")

import numpy as np
import ml_dtypes

import bass_rust
import concourse.bass as bass
import concourse.mybir as mybir
import concourse.tile as tile
from concourse.bass_utils import run_bass_kernel_spmd

F32 = mybir.dt.float32
BF16 = mybir.dt.bfloat16
I32 = mybir.dt.int32
AF = mybir.ActivationFunctionType
ALU = mybir.AluOpType

N_CORES = 8
IMGS_PER_CORE = 2
C = 384
CB = 3          # channel blocks of 128
H = W = 56
PIX = H * W     # 3136
WPAD = 62       # 3 + 56 + 3, even stride
CHUNK = 448     # pixels per chunk (8 rows)
NCHUNK = 7
FD = 1536       # hidden dim
NFC = 12        # hidden blocks of 128
EPS = 1e-6

# conv row-region split: [0, R_PE) tensor engine, [R_PE, R_A) scalar-engine
# products, [R_A, 56) vector-engine products.  All multiples of 8 rows.
R_PE = 24
R_A = 40

MAGIC = 0x5F3759DF

_WAITSPLIT_N = [0]


def _split_waits(nc, max_waits=1):
    """This walrus build rejects instructions with more than one sync-wait
    command; hoist excess waits onto dedicated NoOps on the same engine."""
    for fn in nc.m.functions:
        for bb in fn.blocks:
            insts = bb.instructions
            idx = 0
            while idx < len(insts):
                ins = insts[idx]
                si = ins.sync_info
                if si is not None and len(si.on_wait) > max_waits:
                    waits = list(si.on_wait)
                    extra, keep = waits[:-max_waits], waits[-max_waits:]
                    nops = []
                    for w in extra:
                        _WAITSPLIT_N[0] += 1
                        nops.append(
                            mybir.InstNoOp(
                                name=f"I-wsplit-{_WAITSPLIT_N[0]}",
                                engine=ins.engine,
                                ins=[],
                                outs=[],
                                sync_info=bass_rust.SyncInfo(
                                    on_wait=[w], on_update=[]
                                ),
                            )
                        )
                    ins.sync_info = bass_rust.SyncInfo(
                        on_wait=keep, on_update=list(si.on_update)
                    )
                    insts[idx:idx] = nops
                    idx += len(nops)
                idx += 1


def _build_nc(n_imgs=IMGS_PER_CORE):
    nc = bass.Bass(trn_type="TRN2", target_bir_lowering=False, debug=False)

    xs = nc.dram_tensor("xs", [IMGS_PER_CORE, C, H, W], F32, kind="ExternalInput")
    wt = nc.dram_tensor("wt", [C, 49], F32, kind="ExternalInput")
    dwb = nc.dram_tensor("dwb", [C], F32, kind="ExternalInput")
    w1p = nc.dram_tensor("w1p", [C, FD], BF16, kind="ExternalInput")
    s1n = nc.dram_tensor("s1n", [1, FD], BF16, kind="ExternalInput")
    b1p = nc.dram_tensor("b1p", [FD], F32, kind="ExternalInput")
    w2g = nc.dram_tensor("w2g", [FD, C], BF16, kind="ExternalInput")
    b2g = nc.dram_tensor("b2g", [C], F32, kind="ExternalInput")
    ys = nc.dram_tensor("ys", [IMGS_PER_CORE, C, H, W], F32, kind="ExternalOutput")
    # scratch for the [1,3136] <-> [112,28] rstd transposes
    vscratch = nc.dram_tensor("vscratch", [IMGS_PER_CORE, PIX], F32, kind="Internal")
    rscratch = nc.dram_tensor("rscratch", [IMGS_PER_CORE, PIX], F32, kind="Internal")

    xs3 = xs.ap().rearrange("i c h w -> i c (h w)")
    ys3 = ys.ap().rearrange("i c h w -> i c (h w)")

    with tile.TileContext(nc) as tc:
        with (
            tc.tile_pool(name="const", bufs=1) as constp,
            tc.tile_pool(name="diag", bufs=2) as diagp,
            tc.tile_pool(name="xpad", bufs=6) as xpadp,
            tc.tile_pool(name="xpsh", bufs=6) as xpshp,
            tc.tile_pool(name="acc", bufs=6) as accp,
            tc.tile_pool(name="prod", bufs=5) as prodp,
            tc.tile_pool(name="xt", bufs=2) as xtp,
            tc.tile_pool(name="h", bufs=2) as hp,
            tc.tile_pool(name="small", bufs=2) as smallp,
            tc.tile_pool(name="stat", bufs=2) as statp,
            tc.tile_pool(name="outp", bufs=2) as outp,
            tc.tile_pool(name="ps1", bufs=2, space="PSUM") as ps1p,
            tc.tile_pool(name="ps2", bufs=2, space="PSUM") as ps2p,
            tc.tile_pool(name="psstat", bufs=1, space="PSUM") as psstatp,
            tc.tile_pool(name="psrb", bufs=1, space="PSUM") as psrbp,
            tc.tile_pool(name="pstap", bufs=2, space="PSUM") as pstapp,
        ):
            # ---- static weights ----
            wt_sb = constp.tile([128, CB, 49], F32)
            nc.sync.dma_start(wt_sb[:], wt.ap().rearrange("(cb p) t -> p cb t", p=128))
            wtbf_sb = constp.tile([128, CB, 49], BF16)
            nc.vector.tensor_copy(wtbf_sb[:], wt_sb[:])
            dwb_sb = constp.tile([128, CB], F32)
            nc.sync.dma_start(dwb_sb[:], dwb.ap().rearrange("(cb p) -> p cb", p=128))
            w1_sb = constp.tile([128, CB, FD], BF16)
            nc.sync.dma_start(w1_sb[:], w1p.ap().rearrange("(cb p) f -> p cb f", p=128))
            s1_sb = constp.tile([1, FD], BF16)
            nc.sync.dma_start(s1_sb[:], s1n.ap())
            b1_sb = constp.tile([128, NFC], F32)
            nc.sync.dma_start(b1_sb[:], b1p.ap().rearrange("(fc p) -> p fc", p=128))
            w2_sb = constp.tile([128, NFC, C], BF16)
            nc.sync.dma_start(w2_sb[:], w2g.ap().rearrange("(fc p) c -> p fc c", p=128))
            b2_sb = constp.tile([128, CB], F32)
            nc.sync.dma_start(b2_sb[:], b2g.ap().rearrange("(cb p) -> p cb", p=128))
            ones_bf = constp.tile([128, 128], BF16)
            nc.vector.memset(ones_bf[:], 1.0)

            taps = [(d, e) for d in range(-3, 4) for e in range(-3, 4)
                    if not (d == 0 and e == 0)]

            # ---------------- per-image: conv then chunk-pipelined LN+MLP ----------------
            accs = {}  # (img, cb) -> [acc_pe, acc_a, acc_d]

            def acc_chunk(img, cb, ch):
                if ch < R_PE // 8:
                    t = accs[(img, cb)][0]
                    o = ch
                elif ch < R_A // 8:
                    t = accs[(img, cb)][1]
                    o = ch - R_PE // 8
                else:
                    t = accs[(img, cb)][2]
                    o = ch - R_A // 8
                return t[:, o * CHUNK : (o + 1) * CHUNK]

            for img in range(n_imgs):
                for cb in range(CB):
                    cs = slice(cb * 128, (cb + 1) * 128)
                    diag = diagp.tile([128, 49, 128], BF16, tag="diag")
                    nc.gpsimd.affine_select(
                        out=diag[:],
                        in_=wtbf_sb[:, cb, :, None].to_broadcast((128, 49, 128)),
                        compare_op=ALU.is_equal,
                        fill=0.0,
                        base=0,
                        channel_multiplier=1,
                        pattern=[[0, 49], [-1, 128]],
                    )
                    xpad = xpadp.tile([128, H * WPAD], BF16, tag="xpad")
                    xp3 = xpad.rearrange("p (h w) -> p h w", w=WPAD)
                    nc.gpsimd.memset(xp3[:, :, 0:3], 0.0)
                    nc.gpsimd.memset(xp3[:, :, 59:62], 0.0)
                    nc.gpsimd.dma_start(
                        xp3[:, :, 3:59],
                        xs3[img, cs].rearrange("c (h w) -> c h w", w=W),
                    )
                    # xpsh: 1-element-shifted copy for DVE product alignment;
                    # only rows [R_A-3, H) are ever read by the D region.
                    XSH0 = R_A - 3
                    xpsh = xpshp.tile([128, (H - XSH0) * WPAD], BF16, tag="xpsh")
                    nc.gpsimd.tensor_copy(
                        xpsh[:, 0 : (H - XSH0) * WPAD - 1],
                        xpad[:, XSH0 * WPAD + 1 : H * WPAD],
                    )
                    xps3 = xpsh.rearrange("p (h w) -> p h w", w=WPAD)

                    # --- tensor-engine region: rows [0, R_PE), 448-px psum chunks
                    acc_pe = accp.tile([128, R_PE * W], BF16, tag="acc_pe")
                    for hc0 in range(0, R_PE, 8):
                        pst = pstapp.tile([128, CHUNK], F32, tag="pstap")
                        # center tap first: full coverage, clears the bank
                        nc.tensor.matmul(
                            pst[:], diag[:, 24, :], xp3[:, hc0 : hc0 + 8, 3:59],
                            start=True, stop=False,
                        )
                        for i, (d, e) in enumerate(taps):
                            hs = max(hc0, -d)
                            he = min(hc0 + 8, H - d)
                            if he <= hs:
                                continue
                            t = (d + 3) * 7 + (e + 3)
                            nc.tensor.matmul(
                                pst[:, (hs - hc0) * W : (he - hc0) * W],
                                diag[:, t, :],
                                xp3[:, hs + d : he + d, 3 + e : 3 + e + W],
                                start=False,
                                stop=(i == len(taps) - 1),
                            )
                        nc.scalar.activation(
                            acc_pe[:, hc0 * W : (hc0 + 8) * W], pst[:],
                            AF.Identity, bias=dwb_sb[:, cb : cb + 1],
                        )

                    # --- scalar/vector product regions
                    def prod_region(r0, r1, prod_engine, acc_tag):
                        rows = r1 - r0
                        acc = accp.tile([128, rows * W], BF16, tag=acc_tag)
                        a3 = acc.rearrange("p (h w) -> p h w", w=W)
                        nc.scalar.activation(
                            a3[:], xp3[:, r0:r1, 3:59], AF.Identity,
                            bias=dwb_sb[:, cb : cb + 1],
                            scale=wt_sb[:, cb, 24:25],
                        )
                        for d, e in taps:
                            hs = max(r0, -d)
                            he = min(r1, H - d)
                            if he <= hs:
                                continue
                            t = (d + 3) * 7 + (e + 3)
                            nrow = he - hs
                            prod = prodp.tile([128, (r1 - r0) * W], BF16, tag="prod")
                            p3 = prod.rearrange("p (h w) -> p h w", w=W)
                            if prod_engine is nc.scalar:
                                nc.scalar.activation(
                                    p3[:, :nrow, :],
                                    xp3[:, hs + d : he + d, 3 + e : 3 + e + W],
                                    AF.Identity,
                                    scale=wt_sb[:, cb, t : t + 1],
                                )
                            else:
                                col = 3 + e
                                shifted = col % 2
                                src = xps3 if shifted else xp3
                                roff = (R_A - 3) if shifted else 0
                                col -= shifted
                                nc.vector.tensor_scalar(
                                    p3[:, :nrow, :],
                                    src[:, hs + d - roff : he + d - roff, col : col + W],
                                    wt_sb[:, cb, t : t + 1],
                                    None,
                                    ALU.mult,
                                )
                            nc.vector.tensor_tensor(
                                a3[:, hs - r0 : he - r0, :],
                                a3[:, hs - r0 : he - r0, :],
                                p3[:, :nrow, :],
                                ALU.add,
                            )
                        return acc

                    acc_a = prod_region(R_PE, R_A, nc.scalar, "acc_a")
                    acc_d = prod_region(R_A, H, nc.vector, "acc_d")
                    accs[(img, cb)] = [acc_pe, acc_a, acc_d]

                # ------------- chunk-pipelined LN stats + rstd + MLP -------------
                # High priority: these ops gate the tensor engine's MLP work;
                # without it the scheduler drains all queued conv products
                # first and PE starves.
                from contextlib import ExitStack as _ES
                for ch in range(NCHUNK):
                    _hp = tc.high_priority(offset=500000)
                    _hp.__enter__()
                    sl = slice(ch * CHUNK, (ch + 1) * CHUNK)
                    pmu = psstatp.tile([1, CHUNK], F32, tag="ps_stat")
                    for cb in range(CB):
                        nc.tensor.matmul(
                            pmu[:], ones_bf[:, 0:1], acc_chunk(img, cb, ch),
                            start=(cb == 0), stop=(cb == CB - 1),
                        )
                    mu32 = smallp.tile([1, CHUNK], F32, tag="mu32")
                    nc.vector.tensor_scalar_mul(mu32[:], pmu[:], 1.0 / C)
                    pmsq = psstatp.tile([1, CHUNK], F32, tag="ps_stat")
                    for cb in range(CB):
                        ysq = statp.tile([128, CHUNK], BF16, tag="ysq")
                        nc.scalar.activation(ysq[:], acc_chunk(img, cb, ch), AF.Square)
                        nc.tensor.matmul(
                            pmsq[:], ones_bf[:, 0:1], ysq[:],
                            start=(cb == 0), stop=(cb == CB - 1),
                        )
                    tq = smallp.tile([1, CHUNK], F32, tag="tq")
                    nc.vector.tensor_mul(tq[:], mu32[:], mu32[:])
                    # var = E[y^2] - mu^2
                    vchunk = smallp.tile([1, CHUNK], F32, tag="vchunk")
                    nc.vector.scalar_tensor_tensor(
                        out=vchunk[:], in0=pmsq[:], scalar=1.0 / C, in1=tq[:],
                        op0=ALU.mult, op1=ALU.subtract,
                    )
                    nc.sync.dma_start(vscratch.ap()[img : img + 1, sl], vchunk[0:1, :])

                    # per-chunk Newton rsqrt in [56,8] transposed layout
                    vpf = smallp.tile([56, 8], F32, tag="vpf")
                    nc.sync.dma_start(vpf[:], vscratch.ap()[img, sl].rearrange("(p f) -> p f", p=56))
                    v_eps = smallp.tile([56, 8], F32, tag="veps")
                    nc.vector.tensor_scalar_add(v_eps[:], vpf[:], EPS)
                    yr = smallp.tile([56, 8], F32, tag="yr")
                    ti = smallp.tile([56, 8], I32, tag="ti")
                    nc.vector.tensor_scalar(
                        ti[:], v_eps[:].bitcast(I32), 1, None, ALU.logical_shift_right
                    )
                    nc.vector.tensor_scalar(ti[:], ti[:], 0xFFFFFFFF, None, ALU.bitwise_xor)
                    nc.vector.tensor_scalar(yr[:].bitcast(I32), ti[:], MAGIC + 1, None, ALU.add)
                    rr = smallp.tile([56, 8], F32, tag="rr")
                    for _ in range(3):
                        nc.vector.tensor_mul(rr[:], yr[:], yr[:])
                        nc.vector.tensor_mul(rr[:], rr[:], v_eps[:])
                        nc.vector.tensor_scalar(rr[:], rr[:], -0.5, 1.5, ALU.mult, ALU.add)
                        nc.vector.tensor_mul(yr[:], yr[:], rr[:])
                    nc.sync.dma_start(
                        rscratch.ap()[img, sl].rearrange("(p f) -> p f", p=56), yr[:]
                    )
                    rstd_c = smallp.tile([1, CHUNK], BF16, tag="rstdc")
                    nc.gpsimd.dma_start(rstd_c[0:1, :], rscratch.ap()[img : img + 1, sl])

                    rb = psrbp.tile([128, CHUNK], F32, tag="rb")
                    nc.tensor.matmul(rb[:], ones_bf[0:1, :], rstd_c[0:1, :], start=True, stop=True)
                    mutr = smallp.tile([1, CHUNK], BF16, tag="mutr")
                    nc.vector.tensor_mul(mutr[:], mu32[:], rstd_c[0:1, :])
                    xts = []
                    for cb in range(CB):
                        xt = xtp.tile([128, CHUNK], BF16, tag=f"xt{cb}")
                        nc.vector.tensor_mul(xt[:], acc_chunk(img, cb, ch), rb[:])
                        xts.append(xt)
                    hblk = hp.tile([128, NFC, CHUNK], BF16, tag="h")
                    for fc in range(NFC):
                        fsl = slice(fc * 128, (fc + 1) * 128)
                        p1 = ps1p.tile([128, CHUNK], F32, tag="p1")
                        for cb in range(CB):
                            nc.tensor.matmul(
                                p1[:], w1_sb[:, cb, fsl], xts[cb][:],
                                start=(cb == 0), stop=False,
                            )
                        nc.tensor.matmul(p1[:], s1_sb[:, fsl], mutr[:], start=False, stop=True)
                        nc.scalar.activation(
                            hblk[:, fc, :], p1[:], AF.Gelu, bias=b1_sb[:, fc : fc + 1]
                        )
                    for cb in range(CB):
                        cs = slice(cb * 128, (cb + 1) * 128)
                        p2 = ps2p.tile([128, CHUNK], F32, tag="p2")
                        for fc in range(NFC):
                            nc.tensor.matmul(
                                p2[:], w2_sb[:, fc, cs], hblk[:, fc, :],
                                start=(fc == 0), stop=(fc == NFC - 1),
                            )
                        xres = outp.tile([128, CHUNK], F32, tag="xres")
                        nc.sync.dma_start(xres[:], xs3[img, cs, sl])
                        osb = outp.tile([128, CHUNK], F32, tag="osb")
                        nc.vector.scalar_tensor_tensor(
                            out=osb[:], in0=p2[:], scalar=b2_sb[:, cb : cb + 1],
                            in1=xres[:], op0=ALU.add, op1=ALU.add,
                        )
                        nc.sync.dma_start(ys3[img, cs, sl], osb[:])
                    _hp.__exit__(None, None, None)

    _split_waits(nc)
    return nc


_NC_CACHE = None


def _host_fold(inputs):
    dw_w = np.asarray(inputs["dw_w"], dtype=np.float32)
    dw_b = np.asarray(inputs["dw_b"], dtype=np.float32)
    ln_w = np.asarray(inputs["ln_w"], dtype=np.float32)
    ln_b = np.asarray(inputs["ln_b"], dtype=np.float32)
    w1 = np.asarray(inputs["w1"], dtype=np.float32)
    b1 = np.asarray(inputs["b1"], dtype=np.float32)
    w2 = np.asarray(inputs["w2"], dtype=np.float32)
    b2 = np.asarray(inputs["b2"], dtype=np.float32)
    gamma = np.asarray(inputs["gamma"], dtype=np.float32)

    w1p = (ln_w[:, None] * w1).astype(np.float32)          # LN scale into w1
    b1p = (b1 + ln_b @ w1).astype(np.float32)              # LN shift into b1
    s1n = -(w1p.sum(axis=0, keepdims=True))                # mean-correction row
    w2g = (w2 * gamma[None, :]).astype(np.float32)         # layer-scale into w2
    b2g = (gamma * b2).astype(np.float32)
    wtap = dw_w[:, :, 0, :].transpose(2, 0, 1).reshape(C, 49).copy()

    return {
        "wt": wtap.astype(np.float32),
        "dwb": dw_b.astype(np.float32),
        "w1p": w1p.astype(ml_dtypes.bfloat16),
        "s1n": s1n.astype(ml_dtypes.bfloat16),
        "b1p": b1p.astype(np.float32),
        "w2g": w2g.astype(ml_dtypes.bfloat16),
        "b2g": b2g.astype(np.float32),
    }


def make_in_maps(inputs):
    x = np.asarray(inputs["x"], dtype=np.float32)
    common = _host_fold(inputs)
    in_maps = []
    for k in range(N_CORES):
        m = dict(common)
        m["xs"] = np.ascontiguousarray(x[k * IMGS_PER_CORE : (k + 1) * IMGS_PER_CORE])
        in_maps.append(m)
    return in_maps


def kernel(**inputs):
    global _NC_CACHE
    in_maps = make_in_maps(inputs)
    if _NC_CACHE is None:
        _NC_CACHE = _build_nc()
    res = run_bass_kernel_spmd(_NC_CACHE, in_maps, core_ids=list(range(N_CORES)))
    out = np.concatenate([res.results[k]["ys"] for k in range(N_CORES)], axis=0)
    return out.astype(np.float32)


if __name__ == "__main__":
    rng = np.random.default_rng(0)
    ins = {
        "x": rng.standard_normal((16, C, H, W), dtype=np.float32),
        "dw_w": 0.02 * rng.standard_normal((7, 7, 1, C), dtype=np.float32),
        "dw_b": 0.02 * rng.standard_normal((C,), dtype=np.float32),
        "ln_w": np.ones(C, np.float32),
        "ln_b": np.zeros(C, np.float32),
        "w1": (C**-0.5) * rng.standard_normal((C, FD), dtype=np.float32),
        "b1": 0.02 * rng.standard_normal((FD,), dtype=np.float32),
        "w2": ((4 * C) ** -0.5) * rng.standard_normal((FD, C), dtype=np.float32),
        "b2": 0.02 * rng.standard_normal((C,), dtype=np.float32),
        "gamma": np.full((C,), 1e-6, np.float32),
    }
    out = kernel(**ins)
    print("out", out.shape, out.dtype, np.abs(out).mean())


# revision 23
# speedup vs baseline: 123.5715x; 123.5715x over previous
"""ConvNeXt block kernel for Trainium2 (8 NeuronCores, data-parallel over batch).

Reference semantics (per image):
  y = x + gamma * ( GELU( LN(dwconv7x7(x) + dw_b) @ w1 + b1 ) @ w2 + b2 )
with LN over channels, exact (erf) GELU, NCHW in/out.

Distribution: batch 16 -> 2 images per core across 8 cores. No collectives.

Per-core layout: channels on partitions (3 blocks of 128), pixels on the free
dim.  The depthwise 7x7 conv runs on a W-padded bf16 image and is split by
output rows across three engine strategies:
  rows  0..24: TensorE — 49 diagonal-weight matmuls per 448-pixel PSUM chunk
  rows 24..40: ScalarE products (per-partition scale) + VectorE adds
  rows 40..56: VectorE tensor_scalar products (4x mode) + tensor_tensor adds
The MLP runs on the tensor engine in bf16 with LN folded in: stats via a
ones-matmul, rstd via a magic-constant Newton rsqrt on the vector engine in a
[112,28] transposed layout (DRAM bounce), the mean correction as an extra k=1
matmul row, b1 via the GELU activation bias.  The residual is re-loaded from
DRAM in fp32 and added on the vector engine, so the dominant output term never
passes through bf16.
"""

import sys

sys.path.insert(0, "/opt/trn_rl_repo")

import numpy as np
import ml_dtypes

import bass_rust
import concourse.bass as bass
import concourse.mybir as mybir
import concourse.tile as tile
from concourse.bass_utils import run_bass_kernel_spmd

F32 = mybir.dt.float32
BF16 = mybir.dt.bfloat16
I32 = mybir.dt.int32
AF = mybir.ActivationFunctionType
ALU = mybir.AluOpType

N_CORES = 8
IMGS_PER_CORE = 2
C = 384
CB = 3          # channel blocks of 128
H = W = 56
PIX = H * W     # 3136
WPAD = 62       # 3 + 56 + 3, even stride
CHUNK = 448     # pixels per chunk (8 rows)
NCHUNK = 7
FD = 1536       # hidden dim
NFC = 12        # hidden blocks of 128
EPS = 1e-6

# conv row-region split per image: [0, R_PE) tensor engine, [R_PE, R_A)
# scalar-engine products, [R_A, 56) vector-engine products.  All multiples of
# 8 rows.  The second image leans on the tensor engine: its late chunks would
# otherwise starve PE while ACT/DVE finish their conv chains.
REGIONS = {0: (16, 40), 1: (40, 48)}

MAGIC = 0x5F3759DF

_WAITSPLIT_N = [0]


def _split_waits(nc, max_waits=1):
    """This walrus build rejects instructions with more than one sync-wait
    command; hoist excess waits onto dedicated NoOps on the same engine."""
    for fn in nc.m.functions:
        for bb in fn.blocks:
            insts = bb.instructions
            idx = 0
            while idx < len(insts):
                ins = insts[idx]
                si = ins.sync_info
                if si is not None and len(si.on_wait) > max_waits:
                    waits = list(si.on_wait)
                    extra, keep = waits[:-max_waits], waits[-max_waits:]
                    nops = []
                    for w in extra:
                        _WAITSPLIT_N[0] += 1
                        nops.append(
                            mybir.InstNoOp(
                                name=f"I-wsplit-{_WAITSPLIT_N[0]}",
                                engine=ins.engine,
                                ins=[],
                                outs=[],
                                sync_info=bass_rust.SyncInfo(
                                    on_wait=[w], on_update=[]
                                ),
                            )
                        )
                    ins.sync_info = bass_rust.SyncInfo(
                        on_wait=keep, on_update=list(si.on_update)
                    )
                    insts[idx:idx] = nops
                    idx += len(nops)
                idx += 1


def _build_nc(n_imgs=IMGS_PER_CORE):
    nc = bass.Bass(trn_type="TRN2", target_bir_lowering=False, debug=False)

    xs = nc.dram_tensor("xs", [IMGS_PER_CORE, C, H, W], F32, kind="ExternalInput")
    wt = nc.dram_tensor("wt", [C, 49], F32, kind="ExternalInput")
    dwb = nc.dram_tensor("dwb", [C], F32, kind="ExternalInput")
    w1p = nc.dram_tensor("w1p", [C, FD], BF16, kind="ExternalInput")
    s1n = nc.dram_tensor("s1n", [1, FD], BF16, kind="ExternalInput")
    b1p = nc.dram_tensor("b1p", [FD], F32, kind="ExternalInput")
    w2g = nc.dram_tensor("w2g", [FD, C], BF16, kind="ExternalInput")
    b2g = nc.dram_tensor("b2g", [C], F32, kind="ExternalInput")
    ys = nc.dram_tensor("ys", [IMGS_PER_CORE, C, H, W], F32, kind="ExternalOutput")
    # scratch for the [1,3136] <-> [112,28] rstd transposes
    vscratch = nc.dram_tensor("vscratch", [IMGS_PER_CORE, PIX], F32, kind="Internal")
    rscratch = nc.dram_tensor("rscratch", [IMGS_PER_CORE, PIX], F32, kind="Internal")

    xs3 = xs.ap().rearrange("i c h w -> i c (h w)")
    ys3 = ys.ap().rearrange("i c h w -> i c (h w)")

    with tile.TileContext(nc) as tc:
        with (
            tc.tile_pool(name="const", bufs=1) as constp,
            tc.tile_pool(name="diag", bufs=2) as diagp,
            tc.tile_pool(name="xpad", bufs=6) as xpadp,
            tc.tile_pool(name="xpsh", bufs=6) as xpshp,
            tc.tile_pool(name="acc", bufs=3) as accp,
            tc.tile_pool(name="prod", bufs=3) as prodp,
            tc.tile_pool(name="xt", bufs=2) as xtp,
            tc.tile_pool(name="h", bufs=2) as hp,
            tc.tile_pool(name="small", bufs=2) as smallp,
            tc.tile_pool(name="stat", bufs=2) as statp,
            tc.tile_pool(name="outp", bufs=2) as outp,
            tc.tile_pool(name="ps1", bufs=2, space="PSUM") as ps1p,
            tc.tile_pool(name="ps2", bufs=2, space="PSUM") as ps2p,
            tc.tile_pool(name="psstat", bufs=1, space="PSUM") as psstatp,
            tc.tile_pool(name="psrb", bufs=1, space="PSUM") as psrbp,
            tc.tile_pool(name="pstap", bufs=2, space="PSUM") as pstapp,
        ):
            # ---- static weights ----
            wt_sb = constp.tile([128, CB, 49], F32)
            nc.sync.dma_start(wt_sb[:], wt.ap().rearrange("(cb p) t -> p cb t", p=128))
            wtbf_sb = constp.tile([128, CB, 49], BF16)
            nc.vector.tensor_copy(wtbf_sb[:], wt_sb[:])
            dwb_sb = constp.tile([128, CB], F32)
            nc.sync.dma_start(dwb_sb[:], dwb.ap().rearrange("(cb p) -> p cb", p=128))
            w1_sb = constp.tile([128, CB, FD], BF16)
            nc.sync.dma_start(w1_sb[:], w1p.ap().rearrange("(cb p) f -> p cb f", p=128))
            s1_sb = constp.tile([1, FD], BF16)
            nc.sync.dma_start(s1_sb[:], s1n.ap())
            b1_sb = constp.tile([128, NFC], F32)
            nc.sync.dma_start(b1_sb[:], b1p.ap().rearrange("(fc p) -> p fc", p=128))
            w2_sb = constp.tile([128, NFC, C], BF16)
            nc.sync.dma_start(w2_sb[:], w2g.ap().rearrange("(fc p) c -> p fc c", p=128))
            b2_sb = constp.tile([128, CB], F32)
            nc.sync.dma_start(b2_sb[:], b2g.ap().rearrange("(cb p) -> p cb", p=128))
            ones_bf = constp.tile([128, 128], BF16)
            nc.vector.memset(ones_bf[:], 1.0)

            taps = [(d, e) for d in range(-3, 4) for e in range(-3, 4)
                    if not (d == 0 and e == 0)]

            # ---------------- per-image: conv then chunk-pipelined LN+MLP ----------------
            accs = {}  # (img, cb) -> [acc_pe, acc_a, acc_d]

            def acc_chunk(img, cb, ch):
                R_PE, R_A = REGIONS[img]
                if ch < R_PE // 8:
                    t = accs[(img, cb)][0]
                    o = ch
                elif ch < R_A // 8:
                    t = accs[(img, cb)][1]
                    o = ch - R_PE // 8
                else:
                    t = accs[(img, cb)][2]
                    o = ch - R_A // 8
                return t[:, o * CHUNK : (o + 1) * CHUNK]

            for img in range(n_imgs):
                R_PE, R_A = REGIONS[img]
                for cb in range(CB):
                    cs = slice(cb * 128, (cb + 1) * 128)
                    diag = diagp.tile([128, 49, 128], BF16, tag="diag")
                    nc.gpsimd.affine_select(
                        out=diag[:],
                        in_=wtbf_sb[:, cb, :, None].to_broadcast((128, 49, 128)),
                        compare_op=ALU.is_equal,
                        fill=0.0,
                        base=0,
                        channel_multiplier=1,
                        pattern=[[0, 49], [-1, 128]],
                    )
                    xpad = xpadp.tile([128, H * WPAD], BF16, tag="xpad")
                    xp3 = xpad.rearrange("p (h w) -> p h w", w=WPAD)
                    nc.gpsimd.memset(xp3[:, :, 0:3], 0.0)
                    nc.gpsimd.memset(xp3[:, :, 59:62], 0.0)
                    nc.gpsimd.dma_start(
                        xp3[:, :, 3:59],
                        xs3[img, cs].rearrange("c (h w) -> c h w", w=W),
                    )
                    # xpsh: 1-element-shifted copy for DVE product alignment;
                    # only rows [R_A-3, H) are ever read by the D region.
                    XSH0 = min(REGIONS[i][1] for i in range(n_imgs)) - 3
                    xpsh = xpshp.tile([128, (H - XSH0) * WPAD], BF16, tag="xpsh")
                    nc.gpsimd.tensor_copy(
                        xpsh[:, 0 : (H - XSH0) * WPAD - 1],
                        xpad[:, XSH0 * WPAD + 1 : H * WPAD],
                    )
                    xps3 = xpsh.rearrange("p (h w) -> p h w", w=WPAD)

                    # --- tensor-engine region: rows [0, R_PE), 448-px psum chunks
                    acc_pe = accp.tile([128, R_PE * W], BF16, tag=f"acc_pe{R_PE}")
                    for hc0 in range(0, R_PE, 8):
                        pst = pstapp.tile([128, CHUNK], F32, tag="pstap")
                        # center tap first: full coverage, clears the bank
                        nc.tensor.matmul(
                            pst[:], diag[:, 24, :], xp3[:, hc0 : hc0 + 8, 3:59],
                            start=True, stop=False,
                        )
                        for i, (d, e) in enumerate(taps):
                            hs = max(hc0, -d)
                            he = min(hc0 + 8, H - d)
                            if he <= hs:
                                continue
                            t = (d + 3) * 7 + (e + 3)
                            nc.tensor.matmul(
                                pst[:, (hs - hc0) * W : (he - hc0) * W],
                                diag[:, t, :],
                                xp3[:, hs + d : he + d, 3 + e : 3 + e + W],
                                start=False,
                                stop=(i == len(taps) - 1),
                            )
                        nc.scalar.activation(
                            acc_pe[:, hc0 * W : (hc0 + 8) * W], pst[:],
                            AF.Identity, bias=dwb_sb[:, cb : cb + 1],
                        )

                    # --- scalar/vector product regions
                    def prod_region(r0, r1, prod_engine, acc_tag):
                        rows = r1 - r0
                        acc = accp.tile([128, rows * W], BF16, tag=f"{acc_tag}{rows}")
                        a3 = acc.rearrange("p (h w) -> p h w", w=W)
                        nc.scalar.activation(
                            a3[:], xp3[:, r0:r1, 3:59], AF.Identity,
                            bias=dwb_sb[:, cb : cb + 1],
                            scale=wt_sb[:, cb, 24:25],
                        )
                        for d, e in taps:
                            hs = max(r0, -d)
                            he = min(r1, H - d)
                            if he <= hs:
                                continue
                            t = (d + 3) * 7 + (e + 3)
                            nrow = he - hs
                            prod = prodp.tile([128, (r1 - r0) * W], BF16, tag=f"prod{r1 - r0}")
                            p3 = prod.rearrange("p (h w) -> p h w", w=W)
                            if prod_engine is nc.scalar:
                                nc.scalar.activation(
                                    p3[:, :nrow, :],
                                    xp3[:, hs + d : he + d, 3 + e : 3 + e + W],
                                    AF.Identity,
                                    scale=wt_sb[:, cb, t : t + 1],
                                )
                            else:
                                col = 3 + e
                                shifted = col % 2
                                src = xps3 if shifted else xp3
                                roff = XSH0 if shifted else 0
                                col -= shifted
                                nc.vector.tensor_scalar(
                                    p3[:, :nrow, :],
                                    src[:, hs + d - roff : he + d - roff, col : col + W],
                                    wt_sb[:, cb, t : t + 1],
                                    None,
                                    ALU.mult,
                                )
                            nc.vector.tensor_tensor(
                                a3[:, hs - r0 : he - r0, :],
                                a3[:, hs - r0 : he - r0, :],
                                p3[:, :nrow, :],
                                ALU.add,
                            )
                        return acc

                    acc_a = prod_region(R_PE, R_A, nc.scalar, "acc_a")
                    acc_d = prod_region(R_A, H, nc.vector, "acc_d")
                    accs[(img, cb)] = [acc_pe, acc_a, acc_d]

                # ------------- chunk-pipelined LN stats + rstd + MLP -------------
                # High priority: these ops gate the tensor engine's MLP work;
                # without it the scheduler drains all queued conv products
                # first and PE starves.
                from contextlib import ExitStack as _ES
                for ch in range(NCHUNK):
                    _hp = tc.high_priority(offset=500000)
                    _hp.__enter__()
                    sl = slice(ch * CHUNK, (ch + 1) * CHUNK)
                    pmu = psstatp.tile([1, CHUNK], F32, tag="ps_stat")
                    for cb in range(CB):
                        nc.tensor.matmul(
                            pmu[:], ones_bf[:, 0:1], acc_chunk(img, cb, ch),
                            start=(cb == 0), stop=(cb == CB - 1),
                        )
                    mu32 = smallp.tile([1, CHUNK], F32, tag="mu32")
                    nc.vector.tensor_scalar_mul(mu32[:], pmu[:], 1.0 / C)
                    pmsq = psstatp.tile([1, CHUNK], F32, tag="ps_stat")
                    for cb in range(CB):
                        ysq = statp.tile([128, CHUNK], BF16, tag="ysq")
                        nc.scalar.activation(ysq[:], acc_chunk(img, cb, ch), AF.Square)
                        nc.tensor.matmul(
                            pmsq[:], ones_bf[:, 0:1], ysq[:],
                            start=(cb == 0), stop=(cb == CB - 1),
                        )
                    tq = smallp.tile([1, CHUNK], F32, tag="tq")
                    nc.vector.tensor_mul(tq[:], mu32[:], mu32[:])
                    # var = E[y^2] - mu^2
                    vchunk = smallp.tile([1, CHUNK], F32, tag="vchunk")
                    nc.vector.scalar_tensor_tensor(
                        out=vchunk[:], in0=pmsq[:], scalar=1.0 / C, in1=tq[:],
                        op0=ALU.mult, op1=ALU.subtract,
                    )
                    nc.sync.dma_start(vscratch.ap()[img : img + 1, sl], vchunk[0:1, :])

                    # per-chunk Newton rsqrt in [56,8] transposed layout
                    vpf = smallp.tile([56, 8], F32, tag="vpf")
                    nc.sync.dma_start(vpf[:], vscratch.ap()[img, sl].rearrange("(p f) -> p f", p=56))
                    v_eps = smallp.tile([56, 8], F32, tag="veps")
                    nc.vector.tensor_scalar_add(v_eps[:], vpf[:], EPS)
                    yr = smallp.tile([56, 8], F32, tag="yr")
                    ti = smallp.tile([56, 8], I32, tag="ti")
                    nc.vector.tensor_scalar(
                        ti[:], v_eps[:].bitcast(I32), 1, None, ALU.logical_shift_right
                    )
                    nc.vector.tensor_scalar(ti[:], ti[:], 0xFFFFFFFF, None, ALU.bitwise_xor)
                    nc.vector.tensor_scalar(yr[:].bitcast(I32), ti[:], MAGIC + 1, None, ALU.add)
                    rr = smallp.tile([56, 8], F32, tag="rr")
                    for _ in range(3):
                        nc.vector.tensor_mul(rr[:], yr[:], yr[:])
                        nc.vector.tensor_mul(rr[:], rr[:], v_eps[:])
                        nc.vector.tensor_scalar(rr[:], rr[:], -0.5, 1.5, ALU.mult, ALU.add)
                        nc.vector.tensor_mul(yr[:], yr[:], rr[:])
                    nc.sync.dma_start(
                        rscratch.ap()[img, sl].rearrange("(p f) -> p f", p=56), yr[:]
                    )
                    rstd_c = smallp.tile([1, CHUNK], BF16, tag="rstdc")
                    nc.gpsimd.dma_start(rstd_c[0:1, :], rscratch.ap()[img : img + 1, sl])

                    rb = psrbp.tile([128, CHUNK], F32, tag="rb")
                    nc.tensor.matmul(rb[:], ones_bf[0:1, :], rstd_c[0:1, :], start=True, stop=True)
                    mutr = smallp.tile([1, CHUNK], BF16, tag="mutr")
                    nc.vector.tensor_mul(mutr[:], mu32[:], rstd_c[0:1, :])
                    xts = []
                    for cb in range(CB):
                        xt = xtp.tile([128, CHUNK], BF16, tag=f"xt{cb}")
                        nc.vector.tensor_mul(xt[:], acc_chunk(img, cb, ch), rb[:])
                        xts.append(xt)
                    hblk = hp.tile([128, NFC, CHUNK], BF16, tag="h")
                    for fc in range(NFC):
                        fsl = slice(fc * 128, (fc + 1) * 128)
                        p1 = ps1p.tile([128, CHUNK], F32, tag="p1")
                        for cb in range(CB):
                            nc.tensor.matmul(
                                p1[:], w1_sb[:, cb, fsl], xts[cb][:],
                                start=(cb == 0), stop=False,
                            )
                        nc.tensor.matmul(p1[:], s1_sb[:, fsl], mutr[:], start=False, stop=True)
                        nc.scalar.activation(
                            hblk[:, fc, :], p1[:], AF.Gelu, bias=b1_sb[:, fc : fc + 1]
                        )
                    for cb in range(CB):
                        cs = slice(cb * 128, (cb + 1) * 128)
                        p2 = ps2p.tile([128, CHUNK], F32, tag="p2")
                        for fc in range(NFC):
                            nc.tensor.matmul(
                                p2[:], w2_sb[:, fc, cs], hblk[:, fc, :],
                                start=(fc == 0), stop=(fc == NFC - 1),
                            )
                        xres = outp.tile([128, CHUNK], F32, tag="xres")
                        nc.sync.dma_start(xres[:], xs3[img, cs, sl])
                        osb = outp.tile([128, CHUNK], F32, tag="osb")
                        nc.vector.scalar_tensor_tensor(
                            out=osb[:], in0=p2[:], scalar=b2_sb[:, cb : cb + 1],
                            in1=xres[:], op0=ALU.add, op1=ALU.add,
                        )
                        nc.sync.dma_start(ys3[img, cs, sl], osb[:])
                    _hp.__exit__(None, None, None)

    _split_waits(nc)
    return nc


_NC_CACHE = None


def _host_fold(inputs):
    dw_w = np.asarray(inputs["dw_w"], dtype=np.float32)
    dw_b = np.asarray(inputs["dw_b"], dtype=np.float32)
    ln_w = np.asarray(inputs["ln_w"], dtype=np.float32)
    ln_b = np.asarray(inputs["ln_b"], dtype=np.float32)
    w1 = np.asarray(inputs["w1"], dtype=np.float32)
    b1 = np.asarray(inputs["b1"], dtype=np.float32)
    w2 = np.asarray(inputs["w2"], dtype=np.float32)
    b2 = np.asarray(inputs["b2"], dtype=np.float32)
    gamma = np.asarray(inputs["gamma"], dtype=np.float32)

    w1p = (ln_w[:, None] * w1).astype(np.float32)          # LN scale into w1
    b1p = (b1 + ln_b @ w1).astype(np.float32)              # LN shift into b1
    s1n = -(w1p.sum(axis=0, keepdims=True))                # mean-correction row
    w2g = (w2 * gamma[None, :]).astype(np.float32)         # layer-scale into w2
    b2g = (gamma * b2).astype(np.float32)
    wtap = dw_w[:, :, 0, :].transpose(2, 0, 1).reshape(C, 49).copy()

    return {
        "wt": wtap.astype(np.float32),
        "dwb": dw_b.astype(np.float32),
        "w1p": w1p.astype(ml_dtypes.bfloat16),
        "s1n": s1n.astype(ml_dtypes.bfloat16),
        "b1p": b1p.astype(np.float32),
        "w2g": w2g.astype(ml_dtypes.bfloat16),
        "b2g": b2g.astype(np.float32),
    }


def make_in_maps(inputs):
    x = np.asarray(inputs["x"], dtype=np.float32)
    common = _host_fold(inputs)
    in_maps = []
    for k in range(N_CORES):
        m = dict(common)
        m["xs"] = np.ascontiguousarray(x[k * IMGS_PER_CORE : (k + 1) * IMGS_PER_CORE])
        in_maps.append(m)
    return in_maps


def kernel(**inputs):
    global _NC_CACHE
    in_maps = make_in_maps(inputs)
    if _NC_CACHE is None:
        _NC_CACHE = _build_nc()
    res = run_bass_kernel_spmd(_NC_CACHE, in_maps, core_ids=list(range(N_CORES)))
    out = np.concatenate([res.results[k]["ys"] for k in range(N_CORES)], axis=0)
    return out.astype(np.float32)


if __name__ == "__main__":
    rng = np.random.default_rng(0)
    ins = {
        "x": rng.standard_normal((16, C, H, W), dtype=np.float32),
        "dw_w": 0.02 * rng.standard_normal((7, 7, 1, C), dtype=np.float32),
        "dw_b": 0.02 * rng.standard_normal((C,), dtype=np.float32),
        "ln_w": np.ones(C, np.float32),
        "ln_b": np.zeros(C, np.float32),
        "w1": (C**-0.5) * rng.standard_normal((C, FD), dtype=np.float32),
        "b1": 0.02 * rng.standard_normal((FD,), dtype=np.float32),
        "w2": ((4 * C) ** -0.5) * rng.standard_normal((FD, C), dtype=np.float32),
        "b2": 0.02 * rng.standard_normal((C,), dtype=np.float32),
        "gamma": np.full((C,), 1e-6, np.float32),
    }
    out = kernel(**ins)
    print("out", out.shape, out.dtype, np.abs(out).mean())
